# revision 2
# baseline (speedup 1.0000x reference)
"""GTN (graph transformer network) forward on 8 Trainium2 cores.

Math (mirrors the reference, normalizations folded):
  A[t] = dense adjacency from edge lists           (host, bincount)
  A1 = softmax(w_l0_c1) . A ; A2 = softmax(w_l0_c2) . A ; A3 = softmax(w_l1_c1) . A
  H1 = A1 @ A2                                     (device)
  U  = H1 @ A3                                     (device)
  Because all entries are >= 0 and row scaling commutes through matmul,
  rownorm(rownorm(H1) @ A3) == rownorm(U).  With XW1 = [X @ gcn_w | 1],
  Z = U @ XW1 gives both U @ XW (cols :128) and rowsum(U) (col 128) so the
  row normalization can be applied on the host after the fact.
  y = relu(Z[:, :128]/rowsum + b) -> channel concat -> target gather -> linear.

Sharding: 1D row shard, 512 rows per core, both channels per core.
Device computes, per channel:
  H1T = matmul(lhsT=A2 tiles, rhs=A1_rows^T)   [4096 x 512] (stays in SBUF)
  UT  = matmul(lhsT=A3 tiles, rhs=H1T)         consumed tile-by-tile
  Z  += matmul(lhsT=UT tile,  rhs=XW1 chunk)   accumulated in PSUM over j
All matmuls run in float32r (full-rate fp32 mode, ~1e-4 rel err).
"""

import os
import numpy as np
from contextlib import ExitStack

NUM_EDGE = 5
C = 2
N = 4096
W_IN = 512
W_OUT = 128
NCORES = 8
P = 128
R = N // NCORES          # 512 rows per core
NK = N // P              # 32 chunks of the contraction dims
NI = R // P              # 4 row subtiles per core
DOUT = W_OUT + 4         # 132: XW cols + ones col + zero pad (f32r needs even/4-aligned moving dim)

_NC_CACHE = {}
LAST_EXEC_NS = None


def _build_nc():
    import concourse.tile as tile
    from concourse import bacc, mybir

    nc = bacc.Bacc("TRN2", target_bir_lowering=False, debug=False,
                   num_devices=NCORES)
    f32 = mybir.dt.float32
    f32r = mybir.dt.float32r

    a1t = nc.dram_tensor("a1t", [C, N, R], f32, kind="ExternalInput").ap()
    a2 = nc.dram_tensor("a2", [C, N, N], f32, kind="ExternalInput").ap()
    a3 = nc.dram_tensor("a3", [C, N, N], f32, kind="ExternalInput").ap()
    xw = nc.dram_tensor("xw", [N, DOUT], f32, kind="ExternalInput").ap()
    z = nc.dram_tensor("z", [C, R, DOUT], f32, kind="ExternalOutput").ap()

    with tile.TileContext(nc) as tc, ExitStack() as ctx:
        bigp = ctx.enter_context(tc.tile_pool(name="bigp", bufs=1))
        a1p = ctx.enter_context(tc.tile_pool(name="a1p", bufs=1))
        h1p = ctx.enter_context(tc.tile_pool(name="h1p", bufs=1))
        stripp = ctx.enter_context(tc.tile_pool(name="stripp", bufs=2))
        utp = ctx.enter_context(tc.tile_pool(name="utp", bufs=3))
        zsbp = ctx.enter_context(tc.tile_pool(name="zsbp", bufs=4))
        psH = ctx.enter_context(tc.tile_pool(name="psH", bufs=2, space="PSUM"))
        psU = ctx.enter_context(tc.tile_pool(name="psU", bufs=2, space="PSUM"))
        psZ = ctx.enter_context(tc.tile_pool(name="psZ", bufs=4, space="PSUM"))

        # xw_sb[p, k*DOUT + d] = xw[P*k + p, d]; loaded once, reused by both channels
        xw_sb = bigp.tile([P, NK * DOUT], f32r)
        nc.gpsimd.dma_start(
            xw_sb[:].rearrange("p (k d) -> p k d", k=NK),
            xw.rearrange("(k p) d -> p k d", p=P))

        for c in range(C):
            # a1_sb[p, k*R + i] = A1rowsT[P*k + p, i]
            a1_sb = a1p.tile([P, NK * R], f32r)
            nc.gpsimd.dma_start(
                a1_sb[:].rearrange("p (k i) -> p k i", k=NK),
                a1t[c].rearrange("(k p) i -> p k i", p=P))

            # H1T chunks: h1_sb[p, m*R + i] = H1T[P*m + p, i]
            h1_sb = h1p.tile([P, NK * R], f32r)

            for m in range(NK):
                # strip[p, k*P + j] = a2[c, P*k + p, P*m + j]
                st = stripp.tile([P, NK * P], f32r, tag="strip")
                nc.gpsimd.dma_start(
                    st[:].rearrange("p (k j) -> p k j", k=NK),
                    a2[c][:, m * P:(m + 1) * P].rearrange("(k p) j -> p k j", p=P))
                acc = psH.tile([P, R], f32)
                for k in range(NK):
                    nc.tensor.matmul(acc[:],
                                     st[:, k * P:(k + 1) * P],
                                     a1_sb[:, k * R:(k + 1) * R],
                                     start=(k == 0), stop=(k == NK - 1))
                nc.vector.tensor_copy(h1_sb[:, m * R:(m + 1) * R], acc[:])

            # MM2 (UT tiles) immediately consumed by MM3 (Z accumulation)
            zacc = [psZ.tile([P, DOUT], f32, tag="zacc", name=f"zacc_{c}_{i}")
                    for i in range(NI)]
            for j in range(NK):
                st = stripp.tile([P, NK * P], f32r, tag="strip")
                nc.gpsimd.dma_start(
                    st[:].rearrange("p (k j) -> p k j", k=NK),
                    a3[c][:, j * P:(j + 1) * P].rearrange("(k p) j -> p k j", p=P))
                uacc = psU.tile([P, R], f32)
                for k in range(NK):
                    nc.tensor.matmul(uacc[:],
                                     st[:, k * P:(k + 1) * P],
                                     h1_sb[:, k * R:(k + 1) * R],
                                     start=(k == 0), stop=(k == NK - 1))
                ut = utp.tile([P, R], f32r)
                nc.vector.tensor_copy(ut[:], uacc[:])
                for i in range(NI):
                    nc.tensor.matmul(zacc[i][:],
                                     ut[:, i * P:(i + 1) * P],
                                     xw_sb[:, j * DOUT:(j + 1) * DOUT],
                                     start=(j == 0), stop=(j == NK - 1),
                                     skip_group_check=True)
            for i in range(NI):
                zt = zsbp.tile([P, DOUT], f32)
                nc.vector.tensor_copy(zt[:], zacc[i][:])
                nc.sync.dma_start(z[c, i * P:(i + 1) * P, :], zt[:])

    nc.compile()
    return nc


def _get_nc():
    if "nc" not in _NC_CACHE:
        _NC_CACHE["nc"] = _build_nc()
    return _NC_CACHE["nc"]


def _softmax_rows(w):
    w = np.asarray(w, np.float32)
    e = np.exp(w - w.max(axis=1, keepdims=True))
    return (e / e.sum(axis=1, keepdims=True)).astype(np.float32)


def _install_ntff_hook():
    """Recreate antenv.axon_hooks if the image lacks it (profiling only)."""
    import sys
    import types
    try:
        from antenv.axon_hooks import get_axon_ntff_profile_hook  # noqa: F401
        return
    except ImportError:
        pass
    try:
        from trn_agent_boot.trn_boot import _ntff_profile_via_ctypes
        import antenv
        mod = types.ModuleType("antenv.axon_hooks")
        state = {"h": None}
        mod.set_axon_ntff_profile_hook = lambda h: state.__setitem__("h", h)
        mod.get_axon_ntff_profile_hook = lambda: state["h"]
        sys.modules["antenv.axon_hooks"] = mod
        antenv.axon_hooks = mod
        mod.set_axon_ntff_profile_hook(
            _ntff_profile_via_ctypes("/opt/axon/libaxon_pjrt.so"))
    except Exception:
        pass


def kernel(edge_index, edge_value, X, target_x, w_l0_c1, w_l0_c2, w_l1_c1,
           gcn_w, gcn_b, lin_w, lin_b):
    global LAST_EXEC_NS
    from concourse.bass_utils import run_bass_kernel_spmd

    # dense adjacency stack [NUM_EDGE, N, N], duplicate edges summed
    A = np.empty((NUM_EDGE, N * N), np.float32)
    src = np.asarray(edge_index[:, 0], np.int64)
    dst = np.asarray(edge_index[:, 1], np.int64)
    for t in range(NUM_EDGE):
        flat = src[t] * N + dst[t]
        A[t] = np.bincount(flat, weights=np.asarray(edge_value[t], np.float64),
                           minlength=N * N).astype(np.float32)

    def combo(w):
        f = _softmax_rows(w)               # [C, NUM_EDGE]
        return (f @ A).reshape(C, N, N)    # [C, N, N]

    A1 = combo(w_l0_c1)
    A2 = combo(w_l0_c2)
    A3 = combo(w_l1_c1)
    A = None  # free

    XW = np.asarray(X, np.float32) @ np.asarray(gcn_w, np.float32)   # [N, 128]
    xw1 = np.concatenate([XW, np.ones((N, 1), np.float32),
                      np.zeros((N, 3), np.float32)], axis=1)  # [N, 132]

    in_maps = []
    for ci in range(NCORES):
        rows = slice(ci * R, (ci + 1) * R)
        a1t_c = np.stack([np.ascontiguousarray(A1[c, rows, :].T)
                          for c in range(C)])          # [C, N, R]
        in_maps.append({"a1t": a1t_c, "a2": A2, "a3": A3, "xw": xw1})

    nc = _get_nc()
    _install_ntff_hook()
    trace = bool(int(os.environ.get("GTN_TRACE", "1")))
    import time as _time
    _t0 = _time.time()
    res = run_bass_kernel_spmd(nc, in_maps, list(range(NCORES)), trace=trace)
    _wall_ns = int((_time.time() - _t0) * 1e9)
    LAST_EXEC_NS = res.exec_time_ns if res.exec_time_ns else _wall_ns

    Z = np.concatenate([r["z"] for r in res.results], axis=1)  # [C, N, DOUT]
    s = Z[:, :, W_OUT]                                          # [C, N]
    with np.errstate(divide="ignore", invalid="ignore"):
        sinv = np.where(s == 0, 0.0, 1.0 / s).astype(np.float32)
    Hn = Z[:, :, :W_OUT] * sinv[:, :, None]                     # [C, N, 128]
    Xc = np.maximum(Hn + np.asarray(gcn_b, np.float32)[None, None, :], 0.0)
    X_ = Xc.transpose(1, 0, 2).reshape(N, C * W_OUT)            # [N, 256]
    y = X_[np.asarray(target_x, np.int64)] @ np.asarray(lin_w, np.float32)
    y = y + np.asarray(lin_b, np.float32)
    return y.astype(np.float32)



# revision 7
# speedup vs baseline: 8.7559x; 8.7559x over previous
"""GTN (graph transformer network) forward on 8 Trainium2 cores.

Math (mirrors the reference, normalizations folded):
  A[t] = dense adjacency from edge lists             (host, bincount)
  A1 = softmax(w_l0_c1) . A ; A2 = softmax(w_l0_c2) . A ; A3 = softmax(w_l1_c1) . A
  U  = A1 @ A2 @ A3 per channel.  All entries are >= 0, so row scaling
  commutes through the matmuls and both row normalizations collapse into
  a single rownorm(U).  Only the target rows of U ever reach the output,
  and U only appears as U @ [XW | s*1], so associate right-to-left:
      B  = A3 @ [XW | s*1]          [N, 132]   (rows sharded over cores)
      C2 = A2 @ B                   [N, 132]   (rows sharded over cores)
      Z  = A1[targets] @ C2         [1024,132] (targets sharded over cores)
  Column 128 carries s * rowsum(U) (s = 1/16 keeps fp16 in range), so the
  row normalization is applied on the host after the fact:
      y = relu(Z[:, :128]/(16*Z[:,128]) + b) -> channel concat -> linear.

Sharding: cores own 512-row slabs of B and C2; an 8-core AllGather after
each of the first two stages rebuilds the full [N, 132] operand.  The
host ships A3/A2 slabs pre-transposed (contraction dim on partitions)
and fp16 to halve HBM traffic; all matmuls are fp16 with f32 PSUM.
"""

import os
import numpy as np
from contextlib import ExitStack

NUM_EDGE = 5
C = 2
N = 4096
W_IN = 512
W_OUT = 128
NT = 1024                # targets
NCORES = 8
P = 128
R = N // NCORES          # 512 rows of B/C2 per core
TPC = NT // NCORES       # 128 targets per core
NK = N // P              # 32 contraction chunks
RB = R // P              # 4 row blocks per core
DOUT = W_OUT + 4         # 132: XW cols + scaled-ones col + pad
SSCALE = np.float32(1.0 / 16.0)   # ones-column scale, keeps fp16 in range

_NC_CACHE = {}
LAST_EXEC_NS = None


def _build_nc():
    import concourse.tile as tile
    from concourse import bacc, mybir

    nc = bacc.Bacc("TRN2", target_bir_lowering=False, debug=False,
                   num_devices=NCORES)
    f32 = mybir.dt.float32
    f16 = mybir.dt.float16

    s3 = nc.dram_tensor("s3", [C, N, R], f16, kind="ExternalInput").ap()
    s2 = nc.dram_tensor("s2", [C, N, R], f16, kind="ExternalInput").ap()
    s1 = nc.dram_tensor("s1", [C, N, TPC], f16, kind="ExternalInput").ap()
    xw = nc.dram_tensor("xw", [N, DOUT], f16, kind="ExternalInput").ap()
    z = nc.dram_tensor("z", [C, TPC, DOUT], f32, kind="ExternalOutput").ap()

    groups = [list(range(NCORES))]

    with tile.TileContext(nc) as tc, ExitStack() as ctx:
        slabp = ctx.enter_context(tc.tile_pool(name="slabp", bufs=2))
        s1p = ctx.enter_context(tc.tile_pool(name="s1p", bufs=1))
        xwp = ctx.enter_context(tc.tile_pool(name="xwp", bufs=1))
        rhsp = ctx.enter_context(tc.tile_pool(name="rhsp", bufs=2))
        outp = ctx.enter_context(tc.tile_pool(name="outp", bufs=4))
        zp = ctx.enter_context(tc.tile_pool(name="zp", bufs=2))
        ps = ctx.enter_context(tc.tile_pool(name="ps", bufs=4, space="PSUM"))
        psz = ctx.enter_context(tc.tile_pool(name="psz", bufs=2, space="PSUM"))
        dram = ctx.enter_context(tc.tile_pool(name="dram", bufs=1, space="DRAM"))

        b_in = dram.tile([C * RB * P, DOUT], f16)
        b_out = dram.tile([NCORES * C * RB * P, DOUT], f16, addr_space="Shared")
        c_in = dram.tile([C * RB * P, DOUT], f16)
        c_out = dram.tile([NCORES * C * RB * P, DOUT], f16, addr_space="Shared")

        # xw_sb[p, k*DOUT + d] = xw[P*k + p, d]
        xw_sb = xwp.tile([P, NK * DOUT], f16)
        nc.gpsimd.dma_start(
            xw_sb[:].rearrange("p (k d) -> p k d", k=NK),
            xw.rearrange("(k p) d -> p k d", p=P))

        # A1[targets].T chunks: s1_sb[c][p, k*TPC + t] = A1[tgt_t, P*k+p]
        s1_sb = []
        for c in range(C):
            t = s1p.tile([P, NK * TPC], f16, name=f"s1_{c}")
            nc.gpsimd.dma_start(
                t[:].rearrange("p (k i) -> p k i", k=NK),
                s1[c].rearrange("(k p) i -> p k i", p=P))
            s1_sb.append(t)

        # A3[rows].T slabs (stage-1 stationary operands)
        s3_sb = []
        for c in range(C):
            t = slabp.tile([P, NK * R], f16, tag="slab", name=f"s3_{c}")
            nc.gpsimd.dma_start(
                t[:].rearrange("p (k r) -> p k r", k=NK),
                s3[c].rearrange("(k p) r -> p k r", p=P))
            s3_sb.append(t)

        # stage 1: B[rows_i] = A3[rows_i, :] @ XW1, write to b_in
        for c in range(C):
            for rb in range(RB):
                acc = ps.tile([P, DOUT], f32, tag="acc", name=f"acc1_{c}_{rb}")
                for k in range(NK):
                    nc.tensor.matmul(
                        acc[:],
                        s3_sb[c][:, k * R + rb * P:k * R + (rb + 1) * P],
                        xw_sb[:, k * DOUT:(k + 1) * DOUT],
                        start=(k == 0), stop=(k == NK - 1))
                bt = outp.tile([P, DOUT], f16, tag="bt", name=f"bt_{c}_{rb}")
                nc.vector.tensor_copy(bt[:], acc[:])
                nc.sync.dma_start(
                    b_in[(c * RB + rb) * P:(c * RB + rb + 1) * P, :], bt[:])

        # A2 slabs replace A3 slabs in the rotating pool (stage-2 stationary)
        s2_sb = []
        for c in range(C):
            t = slabp.tile([P, NK * R], f16, tag="slab", name=f"s2_{c}")
            nc.gpsimd.dma_start(
                t[:].rearrange("p (k r) -> p k r", k=NK),
                s2[c].rearrange("(k p) r -> p k r", p=P))
            s2_sb.append(t)

        nc.gpsimd.collective_compute(
            "AllGather", mybir.AluOpType.bypass, replica_groups=groups,
            ins=[b_in[:]], outs=[b_out[:]])

        # b_out flat layout: [core, c, rb, p, d]; chunk k of B[c] is
        # (core=k//RB, rb=k%RB).  Gather per channel into [p, k, d].
        bv = b_out.rearrange("(core c rb p) d -> core c p rb d",
                             core=NCORES, c=C, p=P)
        b_sb = []
        for c in range(C):
            t = rhsp.tile([P, NK * DOUT], f16, tag="brhs", name=f"b_sb_{c}")
            tv = t[:].rearrange("p (core rb d) -> p core rb d",
                                core=NCORES, rb=RB)
            for j in range(NCORES):
                nc.gpsimd.dma_start(tv[:, j], bv[j, c])
            b_sb.append(t)

        # stage 2: C2[rows_i] = A2[rows_i, :] @ B, write to c_in
        for c in range(C):
            for rb in range(RB):
                acc = ps.tile([P, DOUT], f32, tag="acc", name=f"acc2_{c}_{rb}")
                for k in range(NK):
                    nc.tensor.matmul(
                        acc[:],
                        s2_sb[c][:, k * R + rb * P:k * R + (rb + 1) * P],
                        b_sb[c][:, k * DOUT:(k + 1) * DOUT],
                        start=(k == 0), stop=(k == NK - 1))
                ct = outp.tile([P, DOUT], f16, tag="bt", name=f"ct_{c}_{rb}")
                nc.vector.tensor_copy(ct[:], acc[:])
                nc.sync.dma_start(
                    c_in[(c * RB + rb) * P:(c * RB + rb + 1) * P, :], ct[:])

        nc.gpsimd.collective_compute(
            "AllGather", mybir.AluOpType.bypass, replica_groups=groups,
            ins=[c_in[:]], outs=[c_out[:]])

        cv = c_out.rearrange("(core c rb p) d -> core c p rb d",
                             core=NCORES, c=C, p=P)
        # stage 3: Z[targets_i] = A1[targets_i, :] @ C2
        for c in range(C):
            c2t = rhsp.tile([P, NK * DOUT], f16, tag="brhs", name=f"c2_sb_{c}")
            c2v = c2t[:].rearrange("p (core rb d) -> p core rb d",
                                   core=NCORES, rb=RB)
            for j in range(NCORES):
                nc.gpsimd.dma_start(c2v[:, j], cv[j, c])
            acc = psz.tile([P, DOUT], f32, tag="zacc", name=f"zacc_{c}")
            for k in range(NK):
                nc.tensor.matmul(
                    acc[:],
                    s1_sb[c][:, k * TPC:(k + 1) * TPC],
                    c2t[:, k * DOUT:(k + 1) * DOUT],
                    start=(k == 0), stop=(k == NK - 1))
            zt = zp.tile([P, DOUT], f32, tag="zt", name=f"zt_{c}")
            nc.vector.tensor_copy(zt[:], acc[:])
            nc.sync.dma_start(z[c], zt[:])

    nc.compile()
    return nc


def _get_nc():
    if "nc" not in _NC_CACHE:
        _NC_CACHE["nc"] = _build_nc()
    return _NC_CACHE["nc"]


def _softmax_rows(w):
    w = np.asarray(w, np.float32)
    e = np.exp(w - w.max(axis=1, keepdims=True))
    return (e / e.sum(axis=1, keepdims=True)).astype(np.float32)


def _install_ntff_hook():
    """Recreate antenv.axon_hooks if the image lacks it (profiling only)."""
    import sys
    import types
    try:
        from antenv.axon_hooks import get_axon_ntff_profile_hook  # noqa: F401
        return
    except ImportError:
        pass
    try:
        from trn_agent_boot.trn_boot import _ntff_profile_via_ctypes
        import antenv
        mod = types.ModuleType("antenv.axon_hooks")
        state = {"h": None}
        mod.set_axon_ntff_profile_hook = lambda h: state.__setitem__("h", h)
        mod.get_axon_ntff_profile_hook = lambda: state["h"]
        sys.modules["antenv.axon_hooks"] = mod
        antenv.axon_hooks = mod
        mod.set_axon_ntff_profile_hook(
            _ntff_profile_via_ctypes("/opt/axon/libaxon_pjrt.so"))
    except Exception:
        pass


def kernel(edge_index, edge_value, X, target_x, w_l0_c1, w_l0_c2, w_l1_c1,
           gcn_w, gcn_b, lin_w, lin_b):
    global LAST_EXEC_NS
    from concourse.bass_utils import run_bass_kernel_spmd

    # dense adjacency stack [NUM_EDGE, N*N], duplicate edges summed
    A = np.empty((NUM_EDGE, N * N), np.float32)
    src = np.asarray(edge_index[:, 0], np.int64)
    dst = np.asarray(edge_index[:, 1], np.int64)
    for t in range(NUM_EDGE):
        flat = src[t] * N + dst[t]
        A[t] = np.bincount(flat, weights=np.asarray(edge_value[t], np.float64),
                           minlength=N * N).astype(np.float32)

    f2 = _softmax_rows(w_l0_c2)
    f3 = _softmax_rows(w_l1_c1)
    A2 = (f2 @ A).reshape(C, N, N)
    A3 = (f3 @ A).reshape(C, N, N)

    # A1 only at target rows: gather first, then combine
    tgt = np.asarray(target_x, np.int64)
    Asel = A.reshape(NUM_EDGE, N, N)[:, tgt, :]          # [5, NT, N]
    f1 = _softmax_rows(w_l0_c1)
    A1sel = np.einsum("ce,enm->cnm", f1, Asel)            # [C, NT, N]
    A = None
    Asel = None

    XW = (np.asarray(X, np.float32) @ np.asarray(gcn_w, np.float32))
    xw1 = np.concatenate(
        [XW, np.full((N, 1), SSCALE, np.float32), np.zeros((N, 3), np.float32)],
        axis=1).astype(np.float16)                        # [N, 132]

    in_maps = []
    for ci in range(NCORES):
        rows = slice(ci * R, (ci + 1) * R)
        ts = slice(ci * TPC, (ci + 1) * TPC)
        s3_c = np.stack([np.ascontiguousarray(A3[c, rows, :].T.astype(np.float16))
                         for c in range(C)])              # [C, N, R]
        s2_c = np.stack([np.ascontiguousarray(A2[c, rows, :].T.astype(np.float16))
                         for c in range(C)])              # [C, N, R]
        s1_c = np.stack([np.ascontiguousarray(A1sel[c, ts, :].T.astype(np.float16))
                         for c in range(C)])              # [C, N, TPC]
        in_maps.append({"s3": s3_c, "s2": s2_c, "s1": s1_c, "xw": xw1})

    nc = _get_nc()
    _install_ntff_hook()
    trace = bool(int(os.environ.get("GTN_TRACE", "1")))
    import time as _time
    _t0 = _time.time()
    res = run_bass_kernel_spmd(nc, in_maps, list(range(NCORES)), trace=trace)
    _wall_ns = int((_time.time() - _t0) * 1e9)
    LAST_EXEC_NS = res.exec_time_ns if res.exec_time_ns else _wall_ns

    Z = np.concatenate([r["z"] for r in res.results], axis=1)   # [C, NT, DOUT]
    s = Z[:, :, W_OUT] / SSCALE                                 # [C, NT]
    with np.errstate(divide="ignore", invalid="ignore"):
        sinv = np.where(s == 0, 0.0, 1.0 / s).astype(np.float32)
    Hn = Z[:, :, :W_OUT] * sinv[:, :, None]                     # [C, NT, 128]
    Xc = np.maximum(Hn + np.asarray(gcn_b, np.float32)[None, None, :], 0.0)
    X_ = Xc.transpose(1, 0, 2).reshape(NT, C * W_OUT)           # [NT, 256]
    y = X_ @ np.asarray(lin_w, np.float32)
    y = y + np.asarray(lin_b, np.float32)
    return y.astype(np.float32)


# revision 8
# speedup vs baseline: 9.1600x; 1.0461x over previous
"""GTN (graph transformer network) forward on 8 Trainium2 cores.

Math (mirrors the reference, normalizations folded):
  A[t] = dense adjacency from edge lists             (host, bincount)
  A1 = softmax(w_l0_c1) . A ; A2 = softmax(w_l0_c2) . A ; A3 = softmax(w_l1_c1) . A
  U  = A1 @ A2 @ A3 per channel.  All entries are >= 0, so row scaling
  commutes through the matmuls and both row normalizations collapse into
  a single rownorm(U).  Only the target rows of U ever reach the output,
  and U only appears as U @ [XW | s*1], so associate right-to-left:
      B  = A3 @ [XW | s*1]            [N, 132]   (rows sharded over cores)
      G  = A2[rows_i, :] @ B          [512, 132] (per core, stays in SBUF)
      Zp = A1[targets][:, rows_i] @ G [1024,132] (partial over contraction)
  Z = sum_i Zp via one ReduceScatter(add); each core emits its 1/8 chunk.
  Column 128 carries s * rowsum(U) (s = 1/16 keeps fp16 in range), so the
  row normalization is applied on the host after the fact:
      y = relu(Z[:, :128]/(16*Z[:,128]) + b) -> channel concat -> linear.

Device schedule per core (all matmuls fp16 with f32 PSUM):
  dma xw, s3[c0] (in quarters, so stage 1 starts ~5us in)
  stage1 c0 -> AllGather-a; stage1 c1 -> AllGather-b   (split per channel
  so stage 2 of c0 overlaps the second gather)
  stage2 c0 -> stage3 c0 (partial Z, G straight from SBUF) ; same for c1
  ReduceScatter(add) of Zp -> z chunk
"""

import os
import numpy as np
from contextlib import ExitStack

NUM_EDGE = 5
C = 2
N = 4096
W_IN = 512
W_OUT = 128
NT = 1024                # targets
NCORES = 8
P = 128
R = N // NCORES          # 512 rows of B / contraction slab per core
NK = N // P              # 32 contraction chunks
RB = R // P              # 4 row blocks per core
NTB = NT // P            # 8 target blocks
NQ = 4                   # stage-1 slab DMA split (quarters)
KQ = NK // NQ            # 8 chunks per quarter
ZROWS = C * NT // NCORES # 256 rows of the reduce-scattered Z per core
DOUT = W_OUT + 4         # 132: XW cols + scaled-ones col + pad
SSCALE = np.float32(1.0 / 16.0)   # ones-column scale, keeps fp16 in range

_NC_CACHE = {}
LAST_EXEC_NS = None


def _build_nc():
    import concourse.tile as tile
    from concourse import bacc, mybir

    nc = bacc.Bacc("TRN2", target_bir_lowering=False, debug=False,
                   num_devices=NCORES)
    f32 = mybir.dt.float32
    f16 = mybir.dt.float16

    s3 = nc.dram_tensor("s3", [C, N, R], f16, kind="ExternalInput").ap()
    s2 = nc.dram_tensor("s2", [C, N, R], f16, kind="ExternalInput").ap()
    s1 = nc.dram_tensor("s1", [C, R, NT], f16, kind="ExternalInput").ap()
    xw = nc.dram_tensor("xw", [N, DOUT], f16, kind="ExternalInput").ap()
    z = nc.dram_tensor("z", [ZROWS, DOUT], f32, kind="ExternalOutput").ap()

    groups = [list(range(NCORES))]

    with tile.TileContext(nc) as tc, ExitStack() as ctx:
        slabp = ctx.enter_context(tc.tile_pool(name="slabp", bufs=2))
        s1p = ctx.enter_context(tc.tile_pool(name="s1p", bufs=1))
        xwp = ctx.enter_context(tc.tile_pool(name="xwp", bufs=1))
        rhsp = ctx.enter_context(tc.tile_pool(name="rhsp", bufs=2))
        outp = ctx.enter_context(tc.tile_pool(name="outp", bufs=4))
        gp = ctx.enter_context(tc.tile_pool(name="gp", bufs=8))
        zpp = ctx.enter_context(tc.tile_pool(name="zpp", bufs=2))
        ps = ctx.enter_context(tc.tile_pool(name="ps", bufs=4, space="PSUM"))
        ps3 = ctx.enter_context(tc.tile_pool(name="ps3", bufs=4, space="PSUM"))
        dram = ctx.enter_context(tc.tile_pool(name="dram", bufs=1, space="DRAM"))

        b_in = [dram.tile([RB * P, DOUT], f16, name=f"b_in_{c}")
                for c in range(C)]
        b_out = [dram.tile([NCORES * RB * P, DOUT], f16, addr_space="Shared",
                           name=f"b_out_{c}") for c in range(C)]
        rs_in = dram.tile([C * NT, DOUT], f32)
        rs_out = dram.tile([ZROWS, DOUT], f32)

        # xw_sb[p, k*DOUT + d] = xw[P*k + p, d]
        xw_sb = xwp.tile([P, NK * DOUT], f16)
        nc.gpsimd.dma_start(
            xw_sb[:].rearrange("p (k d) -> p k d", k=NK),
            xw.rearrange("(k p) d -> p k d", p=P))

        # A3 row-slab transposed: s3_sb[c][p, k*R + r] = A3[c, rows_i[r], P*k+p]
        # c0 loads in quarters so stage-1 matmuls can start early.
        s3_sb = []
        for c in range(C):
            t = slabp.tile([P, NK * R], f16, tag="slab", name=f"s3_{c}")
            tv = t[:].rearrange("p (k r) -> p k r", k=NK)
            sv = s3[c].rearrange("(k p) r -> p k r", p=P)
            for q in range(NQ):
                nc.gpsimd.dma_start(tv[:, q * KQ:(q + 1) * KQ],
                                    sv[:, q * KQ:(q + 1) * KQ])
            s3_sb.append(t)

        # stage 1: B[rows_i] = A3[rows_i, :] @ XW1, quarter-interleaved so
        # compute on quarter q overlaps the DMA of quarter q+1
        for c in range(C):
            accs = [ps.tile([P, DOUT], f32, tag="acc", name=f"acc1_{c}_{rb}")
                    for rb in range(RB)]
            for q in range(NQ):
                for rb in range(RB):
                    for k in range(q * KQ, (q + 1) * KQ):
                        nc.tensor.matmul(
                            accs[rb][:],
                            s3_sb[c][:, k * R + rb * P:k * R + (rb + 1) * P],
                            xw_sb[:, k * DOUT:(k + 1) * DOUT],
                            start=(k == 0), stop=(k == NK - 1),
                            skip_group_check=True)
            for rb in range(RB):
                bt = outp.tile([P, DOUT], f16, tag="bt", name=f"bt_{c}_{rb}")
                nc.vector.tensor_copy(bt[:], accs[rb][:])
                nc.sync.dma_start(b_in[c][rb * P:(rb + 1) * P, :], bt[:])
            nc.gpsimd.collective_compute(
                "AllGather", mybir.AluOpType.bypass, replica_groups=groups,
                ins=[b_in[c][:]], outs=[b_out[c][:]])

        # A2 slabs replace A3 slabs in the rotating pool (stage-2 stationary)
        s2_sb = []
        for c in range(C):
            t = slabp.tile([P, NK * R], f16, tag="slab", name=f"s2_{c}")
            nc.gpsimd.dma_start(
                t[:].rearrange("p (k r) -> p k r", k=NK),
                s2[c].rearrange("(k p) r -> p k r", p=P))
            s2_sb.append(t)

        # A1[targets] contraction-slab transposed:
        # s1_sb[c][p, rb*NT + t] = A1sel[c, t, rows_i[rb*P + p]]
        s1_sb = []
        for c in range(C):
            t = s1p.tile([P, RB * NT], f16, name=f"s1_{c}")
            nc.gpsimd.dma_start(
                t[:].rearrange("p (rb t) -> p rb t", rb=RB),
                s1[c].rearrange("(rb p) t -> p rb t", p=P))
            s1_sb.append(t)

        zp_sb = []
        for c in range(C):
            # gather full B for channel c: b_sb[p, k*DOUT+d] = B[P*k+p, d]
            bt_ = rhsp.tile([P, NK * DOUT], f16, tag="brhs", name=f"b_sb_{c}")
            tv = bt_[:].rearrange("p (core rb d) -> p core rb d",
                                  core=NCORES, rb=RB)
            bv = b_out[c].rearrange("(core rb p) d -> core p rb d",
                                    core=NCORES, p=P)
            for j in range(NCORES):
                nc.gpsimd.dma_start(tv[:, j], bv[j])

            # stage 2: G = A2[rows_i, :] @ B   (4 row blocks, kept in SBUF)
            gts = []
            for rb in range(RB):
                acc = ps.tile([P, DOUT], f32, tag="acc", name=f"acc2_{c}_{rb}")
                for k in range(NK):
                    nc.tensor.matmul(
                        acc[:],
                        s2_sb[c][:, k * R + rb * P:k * R + (rb + 1) * P],
                        bt_[:, k * DOUT:(k + 1) * DOUT],
                        start=(k == 0), stop=(k == NK - 1))
                gt = gp.tile([P, DOUT], f16, tag="gt", name=f"gt_{c}_{rb}")
                nc.vector.tensor_copy(gt[:], acc[:])
                gts.append(gt)

            # stage 3: Zp = A1sel[:, rows_i] @ G  (partial over this slab)
            zt = zpp.tile([P, NTB * DOUT], f32, tag="zp", name=f"zp_{c}")
            for tb in range(NTB):
                acc = ps3.tile([P, DOUT], f32, tag="acc3", name=f"acc3_{c}_{tb}")
                for rb in range(RB):
                    nc.tensor.matmul(
                        acc[:],
                        s1_sb[c][:, rb * NT + tb * P:rb * NT + (tb + 1) * P],
                        gts[rb][:],
                        start=(rb == 0), stop=(rb == RB - 1))
                nc.vector.tensor_copy(zt[:, tb * DOUT:(tb + 1) * DOUT], acc[:])
            nc.sync.dma_start(
                rs_in[c * NT:(c + 1) * NT, :].rearrange("(tb p) d -> p tb d", p=P),
                zt[:].rearrange("p (tb d) -> p tb d", tb=NTB))
            zp_sb.append(zt)

        nc.gpsimd.collective_compute(
            "ReduceScatter", mybir.AluOpType.add, replica_groups=groups,
            ins=[rs_in[:]], outs=[rs_out[:]])
        nc.gpsimd.dma_start(z[:], rs_out[:])

    nc.compile()
    return nc


def _get_nc():
    if "nc" not in _NC_CACHE:
        _NC_CACHE["nc"] = _build_nc()
    return _NC_CACHE["nc"]


def _softmax_rows(w):
    w = np.asarray(w, np.float32)
    e = np.exp(w - w.max(axis=1, keepdims=True))
    return (e / e.sum(axis=1, keepdims=True)).astype(np.float32)


def _install_ntff_hook():
    """Recreate antenv.axon_hooks if the image lacks it (profiling only)."""
    import sys
    import types
    try:
        from antenv.axon_hooks import get_axon_ntff_profile_hook  # noqa: F401
        return
    except ImportError:
        pass
    try:
        from trn_agent_boot.trn_boot import _ntff_profile_via_ctypes
        import antenv
        mod = types.ModuleType("antenv.axon_hooks")
        state = {"h": None}
        mod.set_axon_ntff_profile_hook = lambda h: state.__setitem__("h", h)
        mod.get_axon_ntff_profile_hook = lambda: state["h"]
        sys.modules["antenv.axon_hooks"] = mod
        antenv.axon_hooks = mod
        mod.set_axon_ntff_profile_hook(
            _ntff_profile_via_ctypes("/opt/axon/libaxon_pjrt.so"))
    except Exception:
        pass


def kernel(edge_index, edge_value, X, target_x, w_l0_c1, w_l0_c2, w_l1_c1,
           gcn_w, gcn_b, lin_w, lin_b):
    global LAST_EXEC_NS
    from concourse.bass_utils import run_bass_kernel_spmd

    # dense adjacency stack [NUM_EDGE, N*N], duplicate edges summed
    A = np.empty((NUM_EDGE, N * N), np.float32)
    src = np.asarray(edge_index[:, 0], np.int64)
    dst = np.asarray(edge_index[:, 1], np.int64)
    for t in range(NUM_EDGE):
        flat = src[t] * N + dst[t]
        A[t] = np.bincount(flat, weights=np.asarray(edge_value[t], np.float64),
                           minlength=N * N).astype(np.float32)

    f2 = _softmax_rows(w_l0_c2)
    f3 = _softmax_rows(w_l1_c1)
    A2 = (f2 @ A).reshape(C, N, N)
    A3 = (f3 @ A).reshape(C, N, N)

    # A1 only at target rows: gather first, then combine
    tgt = np.asarray(target_x, np.int64)
    Asel = A.reshape(NUM_EDGE, N, N)[:, tgt, :]          # [5, NT, N]
    f1 = _softmax_rows(w_l0_c1)
    A1sel = np.einsum("ce,enm->cnm", f1, Asel)            # [C, NT, N]
    A = None
    Asel = None

    XW = (np.asarray(X, np.float32) @ np.asarray(gcn_w, np.float32))
    xw1 = np.concatenate(
        [XW, np.full((N, 1), SSCALE, np.float32), np.zeros((N, 3), np.float32)],
        axis=1).astype(np.float16)                        # [N, 132]

    in_maps = []
    for ci in range(NCORES):
        rows = slice(ci * R, (ci + 1) * R)
        s3_c = np.stack([np.ascontiguousarray(A3[c, rows, :].T.astype(np.float16))
                         for c in range(C)])              # [C, N, R]
        s2_c = np.stack([np.ascontiguousarray(A2[c, rows, :].T.astype(np.float16))
                         for c in range(C)])              # [C, N, R]
        s1_c = np.stack([np.ascontiguousarray(
                             A1sel[c, :, rows].astype(np.float16).T)
                         for c in range(C)])              # [C, R, NT]
        in_maps.append({"s3": s3_c, "s2": s2_c, "s1": s1_c, "xw": xw1})

    nc = _get_nc()
    _install_ntff_hook()
    trace = bool(int(os.environ.get("GTN_TRACE", "1")))
    import time as _time
    _t0 = _time.time()
    res = run_bass_kernel_spmd(nc, in_maps, list(range(NCORES)), trace=trace)
    _wall_ns = int((_time.time() - _t0) * 1e9)
    LAST_EXEC_NS = res.exec_time_ns if res.exec_time_ns else _wall_ns

    Z = np.concatenate([r["z"] for r in res.results],
                       axis=0).reshape(C, NT, DOUT)       # [C, NT, 132]
    s = Z[:, :, W_OUT] / SSCALE                           # [C, NT]
    with np.errstate(divide="ignore", invalid="ignore"):
        sinv = np.where(s == 0, 0.0, 1.0 / s).astype(np.float32)
    Hn = Z[:, :, :W_OUT] * sinv[:, :, None]               # [C, NT, 128]
    Xc = np.maximum(Hn + np.asarray(gcn_b, np.float32)[None, None, :], 0.0)
    X_ = Xc.transpose(1, 0, 2).reshape(NT, C * W_OUT)     # [NT, 256]
    y = X_ @ np.asarray(lin_w, np.float32)
    y = y + np.asarray(lin_b, np.float32)
    return y.astype(np.float32)
